# revision 9
# baseline (speedup 1.0000x reference)
"""ContactMapHead Trainium2 kernel (8-way sharded, Bass/Tile).

Problem shapes (hardcoded): B=2, L=401, D=128.

Math (reference):
  pair[b,i,j,k] = x[b,i,:] @ W_bil[k] @ x[b,j,:] + b_bil[k]
  h  = LayerNorm_k(pair) * ln_g + ln_b
  y  = GELU(h @ W1.T + b1)
  contact[b,i,j] = y @ w2 + b2 ;  out = 0.5*(contact + contact^T)

Host folding:
  - W_bil centered over k  -> pair mean over k == 0, so LN = pair * rsqrt(mean(pair^2)+eps)
  - Wg[k,e] = W1[e,k]*ln_g[k],  cvec[e] = W1 @ ln_b + b1  (rides the GELU bias)
  - b2 and the symmetrization are applied on host (O(L^2), trivial)

Sharding: row-parallel. Core c (of 8) handles batch c//4, rows s..s+101
(s in {0,100,200,300}), padded to M=104 rows.

Device pipeline per core, (k x j) layout, bf16 matmuls, two passes so the
ScalarE activation table never thrashes between Sqrt and Gelu:

  step0:   tmp[i,k,e] = sum_d Xc[i,d] Wc[k,d,e]      (128 MMs, stationary Wc[k])
  pass A (per row):
    pair_ps(k,j) = tmpT_i.T @ xT                     (PE, N=401)
    pair_c(bf16) = pair_ps + bshift                  (ACT Identity+bias / DVE ts, alternating)
    pair2(bf16)  = pair_c * pair_c                   (GpSimd TT)
    var row      = ones.T @ pair2                    (PE, M=1, 4 rows/bank, parts 0/32/64/96)
    per group: var -> SBUF, compact into S_all via DMA
  stats (once): std = Sqrt(S/128+eps) [ACT], s = recip_approx(std) [DVE], bf16,
    write s to DRAM scratch
  pass B (per row):
    srep = partition-broadcast DMA of s row (DRAM -> SBUF 128 x L)
    h    = pair_c * srep                             (DVE TT bf16)
    y_ps = Wg.T @ h                                  (PE, N=401)
    h2   = Gelu(y_ps + cvec)                         (ACT, fused bias)
    row  = w2.T @ h2                                 (PE, M=1, 4 rows/bank)
    per group: rows -> SBUF -> DMA out
"""

import numpy as np
import ml_dtypes

import concourse.bass as bass
import concourse.tile as tile
from concourse import bacc, mybir

B, L, D = 2, 401, 128
NCORES = 8
GROUP = 4
NG_FULL = 26
M_FULL = GROUP * NG_FULL  # 104 padded rows per core
ROWS_VALID = 101
STARTS = (0, 100, 200, 300)

BF16 = mybir.dt.bfloat16
F32 = mybir.dt.float32
npbf16 = ml_dtypes.bfloat16
AF = mybir.ActivationFunctionType
ALU = mybir.AluOpType

SQUARE_ENGINE = "gpsimd"  # gpsimd | vector | scalar


def _p32(ap_tile, nrows, ncols):
    """View of a [128, ...] tile exposing rows at partitions 0,32,64,96."""
    v = ap_tile.rearrange("(a b) f -> a b f", b=32)
    return v[:nrows, 0, :ncols]


def kernel_body(tc, ins, out_ap, ngroups):
    nc = tc.nc
    m = GROUP * ngroups
    from contextlib import ExitStack

    with ExitStack() as es:
        consts = es.enter_context(tc.tile_pool(name="consts", bufs=1))
        sb = es.enter_context(tc.tile_pool(name="sb", bufs=6))
        sbh = es.enter_context(tc.tile_pool(name="sbh", bufs=6))
        ps_big = es.enter_context(tc.tile_pool(name="ps_big", bufs=4, space="PSUM"))
        ps_small = es.enter_context(tc.tile_pool(name="ps_small", bufs=3, space="PSUM"))
        dram = es.enter_context(tc.tile_pool(name="dram", bufs=1, space="DRAM"))

        # ---- constants / inputs to SBUF
        xT_sb = consts.tile([128, L], BF16, tag="xT")
        nc.sync.dma_start(out=xT_sb, in_=ins["xT"])
        XcT_sb = consts.tile([128, m], BF16, tag="XcT")
        nc.sync.dma_start(out=XcT_sb, in_=ins["XcT"])
        bsh_sb = consts.tile([128, 1], F32, tag="bsh")
        nc.sync.dma_start(out=bsh_sb, in_=ins["bsh"])
        cvec_sb = consts.tile([128, 1], F32, tag="cvec")
        nc.sync.dma_start(out=cvec_sb, in_=ins["cvec"])
        Wg_sb = consts.tile([128, D], BF16, tag="Wg")
        nc.sync.dma_start(out=Wg_sb, in_=ins["Wg"])
        w2_sb = consts.tile([128, 1], BF16, tag="w2")
        nc.sync.dma_start(out=w2_sb, in_=ins["w2"])
        ones_sb = consts.tile([128, 1], BF16, tag="ones")
        nc.vector.memset(ones_sb, 1.0)
        eps_sb = consts.tile([128, 1], F32, tag="eps")
        nc.vector.memset(eps_sb, 1e-5)

        # W (d x k*e), 8 chunks so step0 can start before the full 4MB lands
        WCH = 2048
        W_t = []
        for c in range(D * D // WCH):
            wt = consts.tile([128, WCH], BF16, tag=f"W{c}")
            nc.sync.dma_start(out=wt, in_=ins["W"][:, c * WCH : (c + 1) * WCH])
            W_t.append(wt)

        T_sb = consts.tile([128, D * m], BF16, tag="T")  # T[e, k*m + i]

        # pair_c rows persist across the two passes
        prow = [
            consts.tile([128, L], BF16, tag=f"prow{i}", name=f"prow{i}")
            for i in range(m)
        ]

        # stats staging
        S_all = consts.tile([128, L], F32, tag="Sall")  # rows 0..m-1 = var, then s
        S_bf = consts.tile([128, L], BF16, tag="Sbf")
        s_dram = dram.tile([m, L], BF16, tag="sdram")

        # ---- step 0: tmp[i,k,e] for all rows
        KB = max(1, 512 // m)  # k's per psum bank
        while D % KB:
            KB -= 1
        for kb in range(D // KB):
            ps0 = ps_big.tile([128, 512], F32, tag="big")
            for kk in range(KB):
                k = kb * KB + kk
                c, off = divmod(k * D, WCH)
                nc.tensor.matmul(
                    ps0[:, kk * m : kk * m + m],
                    W_t[c][:, off : off + D],
                    XcT_sb[:, :m],
                    start=True,
                    stop=True,
                )
            dst = T_sb[:, kb * KB * m : (kb + 1) * KB * m]
            src = ps0[:, : KB * m]
            if kb % 2 == 0:
                nc.scalar.activation(dst, src, AF.Copy)
            else:
                nc.vector.tensor_copy(dst, src)

        T_k_i = T_sb.rearrange("p (k i) -> p k i", i=m)

        # ---- pass A: pair rows + variance
        for g in range(ngroups):
            var_ps = ps_small.tile([128, 512], F32, tag="small")
            for r in range(GROUP):
                i = GROUP * g + r
                pair_ps = ps_big.tile([128, 512], F32, tag="big")
                nc.tensor.matmul(
                    pair_ps[:, :L],
                    T_k_i[:, :, i],
                    xT_sb[:, :L],
                    start=True,
                    stop=True,
                )
                # pair_c = pair + bshift (PSUM -> SBUF bf16), alternate engines
                if i % 2 == 0:
                    nc.scalar.activation(
                        prow[i], pair_ps[:, :L], AF.Identity, bias=bsh_sb, scale=1.0
                    )
                else:
                    nc.vector.tensor_scalar(
                        prow[i],
                        pair_ps[:, :L],
                        bsh_sb,
                        None,
                        op0=ALU.add,
                    )
                pair2 = sb.tile([128, L], BF16, tag="pair2")
                if SQUARE_ENGINE == "gpsimd":
                    nc.gpsimd.tensor_mul(pair2, prow[i], prow[i])
                elif SQUARE_ENGINE == "vector":
                    nc.vector.tensor_mul(pair2, prow[i], prow[i])
                else:
                    nc.scalar.activation(pair2, prow[i], AF.Square)
                nc.tensor.matmul(
                    var_ps[32 * r : 32 * r + 1, :L],
                    ones_sb,
                    pair2,
                    start=True,
                    stop=True,
                    tile_position=(0, 32 * r),
                )
            # var rows -> SBUF. Engines can't address strided partitions, so
            # copy the full tile (same per-lane cost; lanes between the four
            # 32-strides carry junk) and let the DMA pick out the rows.
            var_sb = sb.tile([128, L], F32, tag="var_sb")
            if g % 2 == 0:
                nc.scalar.activation(var_sb, var_ps[:, :L], AF.Copy)
            else:
                nc.vector.tensor_copy(var_sb, var_ps[:, :L])
            nc.sync.dma_start(
                out=S_all[GROUP * g : GROUP * (g + 1), :L],
                in_=_p32(var_sb, GROUP, L),
            )

        # ---- stats (once): std = sqrt(var/128 + eps); s = 1/std; to DRAM
        nc.scalar.activation(
            S_all[:m, :L],
            S_all[:m, :L],
            AF.Sqrt,
            bias=eps_sb[:m] if m <= 128 else eps_sb,
            scale=1.0 / D,
        )
        nc.vector.reciprocal_approx_fast(S_all[:m, :L], S_all[:m, :L])
        nc.vector.tensor_copy(S_bf[:m, :L], S_all[:m, :L])
        nc.sync.dma_start(out=s_dram[:, :], in_=S_bf[:m, :L])

        # ---- pass B: scale, MLP, contact rows
        for g in range(ngroups):
            out_ps = ps_small.tile([128, 512], F32, tag="small")
            h_tiles = []
            for r in range(GROUP):
                i = GROUP * g + r
                srep = sbh.tile([128, L], BF16, tag="srep")
                row = s_dram[i : i + 1, :]
                bcast = bass.AP(
                    tensor=row.tensor, offset=row.offset, ap=[[0, 128], *row.ap[1:]]
                )
                nc.sync.dma_start(out=srep, in_=bcast)
                h = sbh.tile([128, L], BF16, tag="h")
                nc.vector.tensor_mul(h, prow[i], srep)
                h_tiles.append(h)
            y_tiles = []
            for r in range(GROUP):
                y_ps = ps_big.tile([128, 512], F32, tag="big")
                nc.tensor.matmul(
                    y_ps[:, :L], Wg_sb, h_tiles[r], start=True, stop=True
                )
                y_tiles.append(y_ps)
            h2_tiles = []
            for r in range(GROUP):
                h2 = sbh.tile([128, L], BF16, tag="h2")
                nc.scalar.activation(
                    h2, y_tiles[r][:, :L], AF.Gelu, bias=cvec_sb, scale=1.0
                )
                h2_tiles.append(h2)
            for r in range(GROUP):
                nc.tensor.matmul(
                    out_ps[32 * r : 32 * r + 1, :L],
                    w2_sb,
                    h2_tiles[r],
                    start=True,
                    stop=True,
                    tile_position=(0, 32 * r),
                )
            orow = sb.tile([128, L], F32, tag="orow")
            if g % 2 == 0:
                nc.scalar.activation(orow, out_ps[:, :L], AF.Copy)
            else:
                nc.vector.tensor_copy(orow, out_ps[:, :L])
            nc.sync.dma_start(
                out=out_ap[GROUP * g : GROUP * (g + 1), :],
                in_=_p32(orow, GROUP, L),
            )


def build_nc(ngroups=NG_FULL):
    m = GROUP * ngroups
    nc = bacc.Bacc("TRN2", debug=False)
    ins = {
        "xT": nc.dram_tensor("xT", [D, L], BF16, kind="ExternalInput").ap(),
        "XcT": nc.dram_tensor("XcT", [D, m], BF16, kind="ExternalInput").ap(),
        "W": nc.dram_tensor("W", [D, D * D], BF16, kind="ExternalInput").ap(),
        "bsh": nc.dram_tensor("bsh", [D, 1], F32, kind="ExternalInput").ap(),
        "cvec": nc.dram_tensor("cvec", [D, 1], F32, kind="ExternalInput").ap(),
        "Wg": nc.dram_tensor("Wg", [D, D], BF16, kind="ExternalInput").ap(),
        "w2": nc.dram_tensor("w2", [D, 1], BF16, kind="ExternalInput").ap(),
    }
    out = nc.dram_tensor("out", [m, L], F32, kind="ExternalOutput").ap()
    with tile.TileContext(nc) as tc:
        kernel_body(tc, ins, out, ngroups)
    nc.compile()
    return nc


def host_prep(x, W_bil, b_bil, ln_g, ln_b, W1, b1, w2, b2):
    """Fold weights on host; build the 8 per-core input maps."""
    x = np.asarray(x, np.float32)
    W_bil = np.asarray(W_bil, np.float32)
    b_bil = np.asarray(b_bil, np.float32)
    ln_g = np.asarray(ln_g, np.float32)
    ln_b = np.asarray(ln_b, np.float32)
    W1 = np.asarray(W1, np.float32)
    b1 = np.asarray(b1, np.float32)
    w2 = np.asarray(w2, np.float32)

    Wc = W_bil - W_bil.mean(axis=0, keepdims=True)  # (k,d,e)
    W_host = np.ascontiguousarray(Wc.transpose(1, 0, 2).reshape(D, D * D)).astype(
        npbf16
    )
    bsh = (b_bil - b_bil.mean()).reshape(D, 1).astype(np.float32)
    Wg = np.ascontiguousarray((W1 * ln_g[None, :]).T).astype(npbf16)  # (k, e2)
    cvec = (W1 @ ln_b + b1).reshape(D, 1).astype(np.float32)
    w2c = w2.reshape(D, 1).astype(npbf16)

    xT = [np.ascontiguousarray(x[b].T).astype(npbf16) for b in range(B)]  # (D, L)

    in_maps = []
    for c in range(NCORES):
        b, s = c // 4, STARTS[c % 4]
        xc = np.zeros((M_FULL, D), np.float32)
        xc[:ROWS_VALID] = x[b, s : s + ROWS_VALID]
        in_maps.append(
            {
                "xT": xT[b],
                "XcT": np.ascontiguousarray(xc.T).astype(npbf16),
                "W": W_host,
                "bsh": bsh,
                "cvec": cvec,
                "Wg": Wg,
                "w2": w2c,
            }
        )
    return in_maps


def assemble(results, b2):
    """Gather per-core row blocks into the full symmetrized output."""
    contact = np.empty((B, L, L), np.float32)
    for c in range(NCORES):
        b, s = c // 4, STARTS[c % 4]
        contact[b, s : s + ROWS_VALID, :] = results[c]["out"][:ROWS_VALID]
    contact += np.float32(np.asarray(b2, np.float32).reshape(-1)[0])
    return (0.5 * (contact + contact.transpose(0, 2, 1))).astype(np.float32)


_NC_CACHE = {}


def _get_nc():
    if "nc" not in _NC_CACHE:
        _NC_CACHE["nc"] = build_nc(NG_FULL)
    return _NC_CACHE["nc"]


def run_on_device(in_maps, trace=False):
    from concourse.bass_utils import run_bass_kernel_spmd

    nc = _get_nc()
    return run_bass_kernel_spmd(
        nc, in_maps, core_ids=list(range(NCORES)), trace=trace
    )


def kernel(x, W_bil, b_bil, ln_g, ln_b, W1, b1, w2, b2):
    in_maps = host_prep(x, W_bil, b_bil, ln_g, ln_b, W1, b1, w2, b2)
    res = run_on_device(in_maps, trace=False)
    return assemble(res.results, b2)


# revision 19
# speedup vs baseline: 1.0466x; 1.0466x over previous
"""ContactMapHead Trainium2 kernel (8-way sharded, Bass/Tile).

Problem shapes (hardcoded): B=2, L=401, D=128.

Math (reference):
  pair[b,i,j,k] = x[b,i,:] @ W_bil[k] @ x[b,j,:] + b_bil[k]
  h  = LayerNorm_k(pair) * ln_g + ln_b
  y  = GELU(h @ W1.T + b1)
  contact[b,i,j] = y @ w2 + b2 ;  out = 0.5*(contact + contact^T)

Host folding:
  - W_bil centered over k  -> pair mean over k == 0, so LN = pair * rsqrt(mean(pair^2)+eps)
  - Wg[k,e] = W1[e,k]*ln_g[k],  cvec[e] = W1 @ ln_b + b1  (rides the GELU bias)
  - b2 and the symmetrization are applied on host (O(L^2), trivial)

Sharding: row-parallel. Core c (of 8) handles batch c//4, rows s..s+101
(s in {0,100,200,300}), padded to M=104 rows.

Device pipeline per core, (k x j) layout, bf16 matmuls, two passes so the
ScalarE activation table never thrashes between Sqrt and Gelu:

  step0:   tmp[i,k,e] = sum_d Xc[i,d] Wc[k,d,e]      (128 MMs, stationary Wc[k])
  pass A (per row):
    pair_ps(k,j) = tmpT_i.T @ xT                     (PE, N=401)
    pair_c(bf16) = pair_ps + bshift                  (ACT Identity+bias / DVE ts, alternating)
    pair2(bf16)  = pair_c * pair_c                   (GpSimd TT)
    var row      = ones.T @ pair2                    (PE, M=1, 4 rows/bank, parts 0/32/64/96)
    per group: var -> SBUF, compact into S_all via DMA
  stats (once): std = Sqrt(S/128+eps) [ACT], s = recip_approx(std) [DVE], bf16,
    write s to DRAM scratch
  pass B (per row):
    srep = partition-broadcast DMA of s row (DRAM -> SBUF 128 x L)
    h    = pair_c * srep                             (DVE TT bf16)
    y_ps = Wg.T @ h                                  (PE, N=401)
    h2   = Gelu(y_ps + cvec)                         (ACT, fused bias)
    row  = w2.T @ h2                                 (PE, M=1, 4 rows/bank)
    per group: rows -> SBUF -> DMA out
"""

import numpy as np
import ml_dtypes

import concourse.bass as bass
import concourse.tile as tile
from concourse import bacc, mybir

B, L, D = 2, 401, 128
NCORES = 8
GROUP = 4
NG_FULL = 26
M_FULL = GROUP * NG_FULL  # 104 padded rows per core
ROWS_VALID = 101
STARTS = (0, 100, 200, 300)

BF16 = mybir.dt.bfloat16
F32 = mybir.dt.float32
npbf16 = ml_dtypes.bfloat16
AF = mybir.ActivationFunctionType
ALU = mybir.AluOpType

SQUARE_ENGINE = "gpsimd"  # gpsimd | vector | scalar


def _p32(ap_tile, nrows, ncols):
    """View of a [128, ...] tile exposing rows at partitions 0,32,64,96."""
    v = ap_tile.rearrange("(a b) f -> a b f", b=32)
    return v[:nrows, 0, :ncols]


def kernel_body(tc, ins, out_ap, ngroups):
    nc = tc.nc
    m = GROUP * ngroups
    from contextlib import ExitStack

    with ExitStack() as es:
        consts = es.enter_context(tc.tile_pool(name="consts", bufs=1))
        sb = es.enter_context(tc.tile_pool(name="sb", bufs=6))
        sbh = es.enter_context(tc.tile_pool(name="sbh", bufs=6))
        ps_big = es.enter_context(tc.tile_pool(name="ps_big", bufs=5, space="PSUM"))
        ps_small = es.enter_context(tc.tile_pool(name="ps_small", bufs=3, space="PSUM"))
        dram = es.enter_context(tc.tile_pool(name="dram", bufs=1, space="DRAM"))

        # ---- constants / inputs to SBUF
        xT_sb = consts.tile([128, L], BF16, tag="xT")
        nc.sync.dma_start(out=xT_sb, in_=ins["xT"])
        XcT_sb = consts.tile([128, m], BF16, tag="XcT")
        nc.sync.dma_start(out=XcT_sb, in_=ins["XcT"])
        bsh_sb = consts.tile([128, 1], F32, tag="bsh")
        nc.sync.dma_start(out=bsh_sb, in_=ins["bsh"])
        cvec_sb = consts.tile([128, 1], F32, tag="cvec")
        nc.sync.dma_start(out=cvec_sb, in_=ins["cvec"])
        Wg_sb = consts.tile([128, D], BF16, tag="Wg")
        nc.sync.dma_start(out=Wg_sb, in_=ins["Wg"])
        ones_sb = consts.tile([128, 32], BF16, tag="ones")
        nc.vector.memset(ones_sb, 1.0)
        w2r_sb = consts.tile([128, 32], BF16, tag="w2r")
        nc.sync.dma_start(out=w2r_sb, in_=ins["w2"])
        eps_sb = consts.tile([128, 1], F32, tag="eps")
        nc.vector.memset(eps_sb, 1e-5)

        # W (d x k*e), 8 chunks so step0 can start before the full 4MB lands
        WCH = 2048
        W_t = []
        for c in range(D * D // WCH):
            wt = consts.tile([128, WCH], BF16, tag=f"W{c}")
            nc.sync.dma_start(out=wt, in_=ins["W"][:, c * WCH : (c + 1) * WCH])
            W_t.append(wt)

        T_sb = consts.tile([128, D * m], BF16, tag="T")  # T[e, k*m + i]

        # pair_c rows persist across the two passes
        prow = [
            consts.tile([128, L], BF16, tag=f"prow{i}", name=f"prow{i}")
            for i in range(m)
        ]

        # stats staging
        S_all = consts.tile([128, L], F32, tag="Sall")  # rows 0..m-1 = var, then s
        S_bf = consts.tile([128, L], BF16, tag="Sbf")
        s_dram = dram.tile([m, L], BF16, tag="sdram")

        # ---- step 0: tmp[i,k,e] for all rows
        KB = max(1, 512 // m)  # k's per psum bank
        while D % KB:
            KB -= 1
        for kb in range(D // KB):
            ps0 = ps_big.tile([128, 512], F32, tag="big")
            for kk in range(KB):
                k = kb * KB + kk
                c, off = divmod(k * D, WCH)
                nc.tensor.matmul(
                    ps0[:, kk * m : kk * m + m],
                    W_t[c][:, off : off + D],
                    XcT_sb[:, :m],
                    start=True,
                    stop=True,
                )
            dst = T_sb[:, kb * KB * m : (kb + 1) * KB * m]
            src = ps0[:, : KB * m]
            if kb % 2 == 0:
                nc.scalar.activation(dst, src, AF.Copy)
            else:
                nc.vector.tensor_copy(dst, src)

        T_k_i = T_sb.rearrange("p (k i) -> p k i", i=m)

        LH = 208  # ACT/DVE split point for PSUM->SBUF copies

        def split_copy_bias(dst, src_ps):
            """dst = src + bshift, front half on ScalarE, back half on DVE."""
            nc.scalar.activation(
                dst[:, :LH], src_ps[:, :LH], AF.Identity, bias=bsh_sb, scale=1.0
            )
            nc.vector.tensor_scalar(
                dst[:, LH:L], src_ps[:, LH:L], bsh_sb, None, op0=ALU.add
            )

        def split_copy(dst, src_ps):
            nc.scalar.activation(dst[:, :LH], src_ps[:, :LH], AF.Copy)
            nc.vector.tensor_copy(dst[:, LH:L], src_ps[:, LH:L])

        # ---- pass A: pair rows + variance
        for g in range(ngroups):
            var_ps = ps_small.tile([128, 512], F32, tag="small")
            pair_tiles = []
            for r in range(GROUP):
                i = GROUP * g + r
                pair_ps = ps_big.tile([128, 512], F32, tag="big")
                nc.tensor.matmul(
                    pair_ps[:, :L],
                    T_k_i[:, :, i],
                    xT_sb[:, :L],
                    start=True,
                    stop=True,
                )
                pair_tiles.append(pair_ps)
            pair2s = []
            for r in range(GROUP):
                i = GROUP * g + r
                split_copy_bias(prow[i], pair_tiles[r])
                pair2 = sb.tile([128, L], BF16, tag="pair2")
                if SQUARE_ENGINE == "gpsimd":
                    nc.gpsimd.tensor_mul(pair2, prow[i], prow[i])
                elif SQUARE_ENGINE == "vector":
                    nc.vector.tensor_mul(pair2, prow[i], prow[i])
                else:
                    nc.scalar.activation(pair2, prow[i], AF.Square)
                pair2s.append(pair2)
            for r in range(GROUP):
                # M=32 (ones replicated): same N-bound cost as M=1, but fills
                # the whole col-strip so downstream full-tile copies are
                # fully initialized.
                nc.tensor.matmul(
                    var_ps[32 * r : 32 * (r + 1), :L],
                    ones_sb,
                    pair2s[r],
                    start=True,
                    stop=True,
                    tile_position=(0, 32 * r),
                )
            # var rows -> SBUF. Engines can't address strided partitions, so
            # copy the full tile (same per-lane cost; lanes between the four
            # 32-strides carry junk) and let the DMA pick out the rows.
            var_sb = sb.tile([128, L], F32, tag="var_sb")
            split_copy(var_sb, var_ps)
            nc.sync.dma_start(
                out=S_all[GROUP * g : GROUP * (g + 1), :L],
                in_=_p32(var_sb, GROUP, L),
            )

        # ---- stats (once): std = sqrt(var/128 + eps); s = 1/std; to DRAM
        nc.scalar.activation(
            S_all[:m, :L],
            S_all[:m, :L],
            AF.Sqrt,
            bias=eps_sb[:m] if m <= 128 else eps_sb,
            scale=1.0 / D,
        )
        nc.vector.reciprocal_approx_fast(S_all[:m, :L], S_all[:m, :L])
        nc.vector.tensor_copy(S_bf[:m, :L], S_all[:m, :L])
        nc.sync.dma_start(out=s_dram[:, :], in_=S_bf[:m, :L])

        # ---- pass B: scale, MLP, contact rows
        for g in range(ngroups):
            out_ps = ps_small.tile([128, 512], F32, tag="small")
            h_tiles = []
            for r in range(GROUP):
                i = GROUP * g + r
                srep = sbh.tile([128, L], BF16, tag="srep")
                row = s_dram[i : i + 1, :]
                bcast = bass.AP(
                    tensor=row.tensor, offset=row.offset, ap=[[0, 128], *row.ap[1:]]
                )
                nc.sync.dma_start(out=srep, in_=bcast)
                h = sbh.tile([128, L], BF16, tag="h")
                nc.vector.tensor_mul(h, prow[i], srep)
                h_tiles.append(h)
            y_tiles = []
            for r in range(GROUP):
                y_ps = ps_big.tile([128, 512], F32, tag="big")
                nc.tensor.matmul(
                    y_ps[:, :L], Wg_sb, h_tiles[r], start=True, stop=True
                )
                y_tiles.append(y_ps)
            h2_tiles = []
            for r in range(GROUP):
                h2 = sbh.tile([128, L], BF16, tag="h2")
                nc.scalar.activation(
                    h2, y_tiles[r][:, :L], AF.Gelu, bias=cvec_sb, scale=1.0
                )
                h2_tiles.append(h2)
            for r in range(GROUP):
                nc.tensor.matmul(
                    out_ps[32 * r : 32 * (r + 1), :L],
                    w2r_sb,
                    h2_tiles[r],
                    start=True,
                    stop=True,
                    tile_position=(0, 32 * r),
                )
            orow = sb.tile([128, L], F32, tag="orow")
            split_copy(orow, out_ps)
            nc.sync.dma_start(
                out=out_ap[GROUP * g : GROUP * (g + 1), :],
                in_=_p32(orow, GROUP, L),
            )


def build_nc(ngroups=NG_FULL):
    m = GROUP * ngroups
    nc = bacc.Bacc("TRN2", debug=False)
    ins = {
        "xT": nc.dram_tensor("xT", [D, L], BF16, kind="ExternalInput").ap(),
        "XcT": nc.dram_tensor("XcT", [D, m], BF16, kind="ExternalInput").ap(),
        "W": nc.dram_tensor("W", [D, D * D], BF16, kind="ExternalInput").ap(),
        "bsh": nc.dram_tensor("bsh", [D, 1], F32, kind="ExternalInput").ap(),
        "cvec": nc.dram_tensor("cvec", [D, 1], F32, kind="ExternalInput").ap(),
        "Wg": nc.dram_tensor("Wg", [D, D], BF16, kind="ExternalInput").ap(),
        "w2": nc.dram_tensor("w2", [D, 32], BF16, kind="ExternalInput").ap(),
    }
    out = nc.dram_tensor("out", [m, L], F32, kind="ExternalOutput").ap()
    with tile.TileContext(nc) as tc:
        kernel_body(tc, ins, out, ngroups)
    nc.compile()
    return nc


def host_prep(x, W_bil, b_bil, ln_g, ln_b, W1, b1, w2, b2):
    """Fold weights on host; build the 8 per-core input maps."""
    x = np.asarray(x, np.float32)
    W_bil = np.asarray(W_bil, np.float32)
    b_bil = np.asarray(b_bil, np.float32)
    ln_g = np.asarray(ln_g, np.float32)
    ln_b = np.asarray(ln_b, np.float32)
    W1 = np.asarray(W1, np.float32)
    b1 = np.asarray(b1, np.float32)
    w2 = np.asarray(w2, np.float32)

    Wc = W_bil - W_bil.mean(axis=0, keepdims=True)  # (k,d,e)
    W_host = np.ascontiguousarray(Wc.transpose(1, 0, 2).reshape(D, D * D)).astype(
        npbf16
    )
    bsh = (b_bil - b_bil.mean()).reshape(D, 1).astype(np.float32)
    Wg = np.ascontiguousarray((W1 * ln_g[None, :]).T).astype(npbf16)  # (k, e2)
    cvec = (W1 @ ln_b + b1).reshape(D, 1).astype(np.float32)
    w2c = np.ascontiguousarray(np.repeat(w2.reshape(D, 1), 32, axis=1)).astype(npbf16)

    xT = [np.ascontiguousarray(x[b].T).astype(npbf16) for b in range(B)]  # (D, L)

    in_maps = []
    for c in range(NCORES):
        b, s = c // 4, STARTS[c % 4]
        xc = np.zeros((M_FULL, D), np.float32)
        xc[:ROWS_VALID] = x[b, s : s + ROWS_VALID]
        in_maps.append(
            {
                "xT": xT[b],
                "XcT": np.ascontiguousarray(xc.T).astype(npbf16),
                "W": W_host,
                "bsh": bsh,
                "cvec": cvec,
                "Wg": Wg,
                "w2": w2c,
            }
        )
    return in_maps


def assemble(results, b2):
    """Gather per-core row blocks into the full symmetrized output."""
    contact = np.empty((B, L, L), np.float32)
    for c in range(NCORES):
        b, s = c // 4, STARTS[c % 4]
        contact[b, s : s + ROWS_VALID, :] = results[c]["out"][:ROWS_VALID]
    contact += np.float32(np.asarray(b2, np.float32).reshape(-1)[0])
    return (0.5 * (contact + contact.transpose(0, 2, 1))).astype(np.float32)


_NC_CACHE = {}


def _get_nc():
    if "nc" not in _NC_CACHE:
        _NC_CACHE["nc"] = build_nc(NG_FULL)
    return _NC_CACHE["nc"]


def run_on_device(in_maps, trace=False):
    from concourse.bass_utils import run_bass_kernel_spmd

    nc = _get_nc()
    return run_bass_kernel_spmd(
        nc, in_maps, core_ids=list(range(NCORES)), trace=trace
    )


def kernel(x, W_bil, b_bil, ln_g, ln_b, W1, b1, w2, b2):
    in_maps = host_prep(x, W_bil, b_bil, ln_g, ln_b, W1, b1, w2, b2)
    res = run_on_device(in_maps, trace=False)
    return assemble(res.results, b2)


# revision 23
# speedup vs baseline: 1.2235x; 1.1691x over previous
"""ContactMapHead Trainium2 kernel (8-way sharded, Bass/Tile).

Problem shapes (hardcoded): B=2, L=401, D=128.

Math (reference):
  pair[b,i,j,k] = x[b,i,:] @ W_bil[k] @ x[b,j,:] + b_bil[k]
  h  = LayerNorm_k(pair) * ln_g + ln_b
  y  = GELU(h @ W1.T + b1)
  contact[b,i,j] = y @ w2 + b2 ;  out = 0.5*(contact + contact^T)

Host folding:
  - W_bil centered over k  -> pair mean over k == 0, so LN = pair * rsqrt(mean(pair^2)+eps)
  - Wg[k,e] = W1[e,k]*ln_g[k],  cvec[e] = W1 @ ln_b + b1  (rides the GELU bias)
  - b2 and the symmetrization are applied on host (O(L^2), trivial)

Sharding: row-parallel. Core c (of 8) handles batch c//4, rows s..s+101
(s in {0,100,200,300}), padded to M=104 rows.

Device pipeline per core, (k x j) layout, bf16 matmuls, two passes so the
ScalarE activation table never thrashes between Sqrt and Gelu:

  step0:   tmp[i,k,e] = sum_d Xc[i,d] Wc[k,d,e]      (128 MMs, stationary Wc[k])
  pass A (per row):
    pair_ps(k,j) = tmpT_i.T @ xT                     (PE, N=401)
    pair_c(bf16) = pair_ps + bshift                  (ACT Identity+bias / DVE ts, alternating)
    pair2(bf16)  = pair_c * pair_c                   (GpSimd TT)
    var row      = ones.T @ pair2                    (PE, M=1, 4 rows/bank, parts 0/32/64/96)
    per group: var -> SBUF, compact into S_all via DMA
  stats (once): std = Sqrt(S/128+eps) [ACT], s = recip_approx(std) [DVE], bf16,
    write s to DRAM scratch
  pass B (per row):
    srep = partition-broadcast DMA of s row (DRAM -> SBUF 128 x L)
    h    = pair_c * srep                             (DVE TT bf16)
    y_ps = Wg.T @ h                                  (PE, N=401)
    h2   = Gelu(y_ps + cvec)                         (ACT, fused bias)
    row  = w2.T @ h2                                 (PE, M=1, 4 rows/bank)
    per group: rows -> SBUF -> DMA out
"""

import numpy as np
import ml_dtypes

import concourse.bass as bass
import concourse.tile as tile
from concourse import bacc, mybir

B, L, D = 2, 401, 128
NCORES = 8
GROUP = 4
NG_FULL = 26
M_FULL = GROUP * NG_FULL  # 104 padded rows per core
ROWS_VALID = 101
STARTS = (0, 100, 200, 300)

BF16 = mybir.dt.bfloat16
F32 = mybir.dt.float32
npbf16 = ml_dtypes.bfloat16
AF = mybir.ActivationFunctionType
ALU = mybir.AluOpType

SQUARE_ENGINE = "gpsimd"  # gpsimd | vector | scalar


def _p32(ap_tile, nrows, ncols):
    """View of a [128, ...] tile exposing rows at partitions 0,32,64,96."""
    v = ap_tile.rearrange("(a b) f -> a b f", b=32)
    return v[:nrows, 0, :ncols]


def kernel_body(tc, ins, out_ap, ngroups):
    nc = tc.nc
    m = GROUP * ngroups
    from contextlib import ExitStack

    with ExitStack() as es:
        consts = es.enter_context(tc.tile_pool(name="consts", bufs=1))
        sb = es.enter_context(tc.tile_pool(name="sb", bufs=6))
        sbh = es.enter_context(tc.tile_pool(name="sbh", bufs=6))
        ps_big = es.enter_context(tc.tile_pool(name="ps_big", bufs=5, space="PSUM"))
        ps_small = es.enter_context(tc.tile_pool(name="ps_small", bufs=3, space="PSUM"))
        dram = es.enter_context(tc.tile_pool(name="dram", bufs=1, space="DRAM"))

        # ---- constants / inputs to SBUF
        xT_sb = consts.tile([128, L], BF16, tag="xT")
        nc.sync.dma_start(out=xT_sb, in_=ins["xT"])
        XcT_sb = consts.tile([128, m], BF16, tag="XcT")
        nc.sync.dma_start(out=XcT_sb, in_=ins["XcT"])
        bsh_sb = consts.tile([128, 1], F32, tag="bsh")
        nc.sync.dma_start(out=bsh_sb, in_=ins["bsh"])
        cvec_sb = consts.tile([128, 1], F32, tag="cvec")
        nc.sync.dma_start(out=cvec_sb, in_=ins["cvec"])
        Wg_sb = consts.tile([128, D], BF16, tag="Wg")
        nc.sync.dma_start(out=Wg_sb, in_=ins["Wg"])
        ones_sb = consts.tile([128, 32], BF16, tag="ones")
        nc.vector.memset(ones_sb, 1.0)
        w2r_sb = consts.tile([128, 32], BF16, tag="w2r")
        nc.sync.dma_start(out=w2r_sb, in_=ins["w2"])
        eps_sb = consts.tile([128, 1], F32, tag="eps")
        nc.vector.memset(eps_sb, 1e-5)

        # W (d x k*e), 8 chunks so step0 can start before the full 4MB lands
        WCH = 2048
        W_t = []
        for c in range(D * D // WCH):
            wt = consts.tile([128, WCH], BF16, tag=f"W{c}")
            nc.sync.dma_start(out=wt, in_=ins["W"][:, c * WCH : (c + 1) * WCH])
            W_t.append(wt)

        T_sb = consts.tile([128, D * m], BF16, tag="T")  # T[e, k*m + i]

        # pair_c rows persist across the two passes
        prow = [
            consts.tile([128, L], BF16, tag=f"prow{i}", name=f"prow{i}")
            for i in range(m)
        ]

        # stats staging
        S_all = consts.tile([128, L], F32, tag="Sall")  # rows 0..m-1 = var, then s
        S_bf = consts.tile([128, L], BF16, tag="Sbf")
        s_dram = dram.tile([m, L], BF16, tag="sdram")

        # ---- step 0: tmp[i,k,e] for all rows
        KB = max(1, 512 // m)  # k's per psum bank
        while D % KB:
            KB -= 1
        for kb in range(D // KB):
            ps0 = ps_big.tile([128, 512], F32, tag="big")
            for kk in range(KB):
                k = kb * KB + kk
                c, off = divmod(k * D, WCH)
                nc.tensor.matmul(
                    ps0[:, kk * m : kk * m + m],
                    W_t[c][:, off : off + D],
                    XcT_sb[:, :m],
                    start=True,
                    stop=True,
                )
            dst = T_sb[:, kb * KB * m : (kb + 1) * KB * m]
            src = ps0[:, : KB * m]
            if kb % 2 == 0:
                nc.scalar.activation(dst, src, AF.Copy)
            else:
                nc.vector.tensor_copy(dst, src)

        T_k_i = T_sb.rearrange("p (k i) -> p k i", i=m)

        LH = 208  # ACT/DVE split point for PSUM->SBUF copies

        def split_copy_bias(dst, src_ps):
            """dst = src + bshift, front half on ScalarE, back half on DVE."""
            nc.scalar.activation(
                dst[:, :LH], src_ps[:, :LH], AF.Identity, bias=bsh_sb, scale=1.0
            )
            nc.vector.tensor_scalar(
                dst[:, LH:L], src_ps[:, LH:L], bsh_sb, None, op0=ALU.add
            )

        def split_copy(dst, src_ps):
            nc.scalar.activation(dst[:, :LH], src_ps[:, :LH], AF.Copy)
            nc.vector.tensor_copy(dst[:, LH:L], src_ps[:, LH:L])

        # ---- pass A: pair rows + variance
        for g in range(ngroups):
            var_ps = ps_small.tile([128, 512], F32, tag="small")
            pair_tiles = []
            for r in range(GROUP):
                i = GROUP * g + r
                pair_ps = ps_big.tile([128, 512], F32, tag="big")
                nc.tensor.matmul(
                    pair_ps[:, :L],
                    T_k_i[:, :, i],
                    xT_sb[:, :L],
                    start=True,
                    stop=True,
                )
                pair_tiles.append(pair_ps)
            pair2s = []
            for r in range(GROUP):
                i = GROUP * g + r
                split_copy_bias(prow[i], pair_tiles[r])
                pair2 = sb.tile([128, L], BF16, tag="pair2")
                if r % 2 == 0:
                    nc.gpsimd.tensor_mul(pair2, prow[i], prow[i])
                else:
                    nc.vector.tensor_mul(pair2, prow[i], prow[i])
                pair2s.append(pair2)
            for r in range(GROUP):
                # M=32 (ones replicated): same N-bound cost as M=1, but fills
                # the whole col-strip so downstream full-tile copies are
                # fully initialized.
                nc.tensor.matmul(
                    var_ps[32 * r : 32 * (r + 1), :L],
                    ones_sb,
                    pair2s[r],
                    start=True,
                    stop=True,
                    tile_position=(0, 32 * r),
                )
            # var rows -> SBUF. Engines can't address strided partitions, so
            # copy the full tile (same per-lane cost; lanes between the four
            # 32-strides carry junk) and let the DMA pick out the rows.
            var_sb = sb.tile([128, L], F32, tag="var_sb")
            split_copy(var_sb, var_ps)
            nc.gpsimd.dma_start(
                out=S_all[GROUP * g : GROUP * (g + 1), :L],
                in_=_p32(var_sb, GROUP, L),
            )

        # ---- stats (once): std = sqrt(var/128 + eps); s = 1/std; to DRAM
        nc.scalar.activation(
            S_all[:m, :L],
            S_all[:m, :L],
            AF.Sqrt,
            bias=eps_sb[:m] if m <= 128 else eps_sb,
            scale=1.0 / D,
        )
        nc.vector.reciprocal_approx_fast(S_all[:m, :L], S_all[:m, :L])
        nc.vector.tensor_copy(S_bf[:m, :L], S_all[:m, :L])
        nc.sync.dma_start(out=s_dram[:, :], in_=S_bf[:m, :L])

        # ---- pass B: scale, MLP, contact rows
        for g in range(ngroups):
            out_ps = ps_small.tile([128, 512], F32, tag="small")
            # one partition-broadcast DMA for the whole group's s rows
            srep4 = sbh.tile([128, GROUP * L], BF16, tag="srep")
            rows = s_dram[GROUP * g : GROUP * (g + 1), :]
            bcast = bass.AP(
                tensor=rows.tensor, offset=rows.offset, ap=[[0, 128], [1, GROUP * L]]
            )
            nc.sync.dma_start(out=srep4, in_=bcast)
            h_tiles = []
            for r in range(GROUP):
                i = GROUP * g + r
                h = sbh.tile([128, L], BF16, tag="h")
                nc.vector.tensor_mul(h, prow[i], srep4[:, r * L : (r + 1) * L])
                h_tiles.append(h)
            y_tiles = []
            for r in range(GROUP):
                y_ps = ps_big.tile([128, 512], F32, tag="big")
                nc.tensor.matmul(
                    y_ps[:, :L], Wg_sb, h_tiles[r], start=True, stop=True
                )
                y_tiles.append(y_ps)
            h2_tiles = []
            for r in range(GROUP):
                h2 = sbh.tile([128, L], BF16, tag="h2")
                nc.scalar.activation(
                    h2, y_tiles[r][:, :L], AF.Gelu, bias=cvec_sb, scale=1.0
                )
                h2_tiles.append(h2)
            for r in range(GROUP):
                nc.tensor.matmul(
                    out_ps[32 * r : 32 * (r + 1), :L],
                    w2r_sb,
                    h2_tiles[r],
                    start=True,
                    stop=True,
                    tile_position=(0, 32 * r),
                )
            orow = sb.tile([128, L], F32, tag="orow")
            split_copy(orow, out_ps)
            nc.gpsimd.dma_start(
                out=out_ap[GROUP * g : GROUP * (g + 1), :],
                in_=_p32(orow, GROUP, L),
            )


def build_nc(ngroups=NG_FULL):
    m = GROUP * ngroups
    nc = bacc.Bacc("TRN2", debug=False)
    ins = {
        "xT": nc.dram_tensor("xT", [D, L], BF16, kind="ExternalInput").ap(),
        "XcT": nc.dram_tensor("XcT", [D, m], BF16, kind="ExternalInput").ap(),
        "W": nc.dram_tensor("W", [D, D * D], BF16, kind="ExternalInput").ap(),
        "bsh": nc.dram_tensor("bsh", [D, 1], F32, kind="ExternalInput").ap(),
        "cvec": nc.dram_tensor("cvec", [D, 1], F32, kind="ExternalInput").ap(),
        "Wg": nc.dram_tensor("Wg", [D, D], BF16, kind="ExternalInput").ap(),
        "w2": nc.dram_tensor("w2", [D, 32], BF16, kind="ExternalInput").ap(),
    }
    out = nc.dram_tensor("out", [m, L], F32, kind="ExternalOutput").ap()
    with tile.TileContext(nc) as tc:
        kernel_body(tc, ins, out, ngroups)
    nc.compile()
    return nc


def host_prep(x, W_bil, b_bil, ln_g, ln_b, W1, b1, w2, b2):
    """Fold weights on host; build the 8 per-core input maps."""
    x = np.asarray(x, np.float32)
    W_bil = np.asarray(W_bil, np.float32)
    b_bil = np.asarray(b_bil, np.float32)
    ln_g = np.asarray(ln_g, np.float32)
    ln_b = np.asarray(ln_b, np.float32)
    W1 = np.asarray(W1, np.float32)
    b1 = np.asarray(b1, np.float32)
    w2 = np.asarray(w2, np.float32)

    Wc = W_bil - W_bil.mean(axis=0, keepdims=True)  # (k,d,e)
    W_host = np.ascontiguousarray(Wc.transpose(1, 0, 2).reshape(D, D * D)).astype(
        npbf16
    )
    bsh = (b_bil - b_bil.mean()).reshape(D, 1).astype(np.float32)
    Wg = np.ascontiguousarray((W1 * ln_g[None, :]).T).astype(npbf16)  # (k, e2)
    cvec = (W1 @ ln_b + b1).reshape(D, 1).astype(np.float32)
    w2c = np.ascontiguousarray(np.repeat(w2.reshape(D, 1), 32, axis=1)).astype(npbf16)

    xT = [np.ascontiguousarray(x[b].T).astype(npbf16) for b in range(B)]  # (D, L)

    in_maps = []
    for c in range(NCORES):
        b, s = c // 4, STARTS[c % 4]
        xc = np.zeros((M_FULL, D), np.float32)
        xc[:ROWS_VALID] = x[b, s : s + ROWS_VALID]
        in_maps.append(
            {
                "xT": xT[b],
                "XcT": np.ascontiguousarray(xc.T).astype(npbf16),
                "W": W_host,
                "bsh": bsh,
                "cvec": cvec,
                "Wg": Wg,
                "w2": w2c,
            }
        )
    return in_maps


def assemble(results, b2):
    """Gather per-core row blocks into the full symmetrized output."""
    contact = np.empty((B, L, L), np.float32)
    for c in range(NCORES):
        b, s = c // 4, STARTS[c % 4]
        contact[b, s : s + ROWS_VALID, :] = results[c]["out"][:ROWS_VALID]
    contact += np.float32(np.asarray(b2, np.float32).reshape(-1)[0])
    return (0.5 * (contact + contact.transpose(0, 2, 1))).astype(np.float32)


_NC_CACHE = {}


def _get_nc():
    if "nc" not in _NC_CACHE:
        _NC_CACHE["nc"] = build_nc(NG_FULL)
    return _NC_CACHE["nc"]


def run_on_device(in_maps, trace=False):
    from concourse.bass_utils import run_bass_kernel_spmd

    nc = _get_nc()
    return run_bass_kernel_spmd(
        nc, in_maps, core_ids=list(range(NCORES)), trace=trace
    )


def kernel(x, W_bil, b_bil, ln_g, ln_b, W1, b1, w2, b2):
    in_maps = host_prep(x, W_bil, b_bil, ln_g, ln_b, W1, b1, w2, b2)
    res = run_on_device(in_maps, trace=False)
    return assemble(res.results, b2)
